# revision 1
# baseline (speedup 1.0000x reference)
"""AdderNet CNN (6x adder_conv + sync-BN + ReLU6) on 8 Trainium2 NeuronCores.

Sharding: data-parallel over batch (2 images/core), sync-BN via tiny
AllReduce of per-channel (sum, sumsq) per layer.

Per-layer compute (per core):
  - contraction dim k lives on SBUF partitions (128-sized k-blocks)
  - abs-diff feed split ~38/62 between ScalarE (activation Abs with
    per-partition bias = -w) and VectorE (tensor_scalar add + uint16
    bitwise_and 0x7FFF sign-clear), both writing bf16 scratch tiles
  - k-partition-sum via TensorE matmul with a sliding one-hot [128,32]
    stationary matrix: channel o accumulates into PSUM row o%128 of a
    shared per-chunk tile (PE array tiling tile_position=(0,32*quad);
    first matmul of each 32-row quadrant resets it via start=True), so a
    full 128-channel block evacuates with one aligned [128,N] DVE op
  - per-channel sum/sumsq via tensor_scalar/activation(Square) accum_out,
    AllReduce of stats (sync-BN), then BN+ReLU6 via tensor_scalar
    (mult,add then max,min), writing bf16 activations in parity-split
    padded layouts so stride-2 consumers read unit-stride, even-length,
    4B-aligned rows (keeps DVE 4x / clean PE rhs APs).
"""

import sys
import numpy as np

if "/opt/trn_rl_repo" not in sys.path:
    sys.path.insert(0, "/opt/trn_rl_repo")

import ml_dtypes

N_CORES = 8
N_LOC = 2  # images per core
EPS = 1e-5

# (Cin, Cout, k, stride, pad, Hi, Wi, Ho, Wo)
LAYER_SHAPES = [
    (512, 256, 1, 1, 0, 38, 38, 38, 38),
    (256, 512, 3, 2, 1, 38, 38, 19, 19),
    (512, 128, 1, 1, 0, 19, 19, 19, 19),
    (128, 256, 3, 2, 1, 19, 19, 10, 10),
    (256, 128, 1, 1, 0, 10, 10, 10, 10),
    (128, 256, 3, 2, 0, 10, 10, 4, 4),
]

_NC_CACHE = {}


def _host_prep(inputs, shapes=LAYER_SHAPES, n_total=16):
    """Build per-core in_maps from the raw reference inputs."""
    bf16 = ml_dtypes.bfloat16
    x = np.asarray(inputs["x"], dtype=np.float32)
    maps_common = {}
    for li, (Cin, Cout, k, *_rest) in enumerate(shapes, start=1):
        w = np.asarray(inputs[f"w{li}"], dtype=np.float32)  # [Cout,Cin,k,k]
        CB = (Cin + 127) // 128
        if k == 1:
            wt = w[:, :, 0, 0].T.reshape(CB, 128, Cout)
        else:
            # kb = cb*9 + dy*3 + dx
            wt = (
                w.transpose(1, 2, 3, 0)  # [Cin, k, k, Cout]
                .reshape(CB, 128, k * k, Cout)
                .transpose(0, 2, 1, 3)  # [CB, k*k, 128, Cout]
                .reshape(CB * k * k, 128, Cout)
            )
        # negated: device computes |x + w'| with w' = -w
        maps_common[f"w{li}t"] = np.ascontiguousarray(-wt, dtype=np.float32)
        OB = (Cout + 127) // 128
        g = np.asarray(inputs[f"g{li}"], dtype=np.float32).reshape(OB, 128).T
        b = np.asarray(inputs[f"b{li}"], dtype=np.float32).reshape(OB, 128).T
        maps_common[f"g{li}"] = np.ascontiguousarray(g)
        maps_common[f"b{li}"] = np.ascontiguousarray(b)

    Cin0, _, _, _, _, Hi, Wi, _, _ = shapes[0]
    CB0 = (Cin0 + 127) // 128
    in_maps = []
    n_cores = n_total // N_LOC
    for c in range(n_cores):
        xs = x[c * N_LOC : (c + 1) * N_LOC]  # [2, Cin, H, W]
        xt = (
            xs.transpose(1, 0, 2, 3)
            .reshape(CB0, 128, N_LOC * Hi * Wi)
            .astype(bf16)
        )
        m = dict(maps_common)
        m["xt"] = np.ascontiguousarray(xt)
        in_maps.append(m)
    return in_maps


def _build_nc(shapes=LAYER_SHAPES, n_total=16, n_cores=N_CORES, mock_cc=False):
    import concourse.bacc as bacc
    import concourse.mybir as mybir
    from concourse import tile

    OP = mybir.AluOpType
    AF = mybir.ActivationFunctionType
    BF16 = mybir.dt.bfloat16
    F32 = mybir.dt.float32
    U16 = mybir.dt.uint16

    nc = bacc.Bacc(
        "TRN2",
        target_bir_lowering=False,
        debug=False,
        enable_asserts=False,
        num_devices=n_cores,
    )

    # ---------------- derived per-layer geometry ----------------
    # Activation storage layouts (per layer output), bf16:
    #  L1 -> parity-split padded 40x40 grid: [128, OB, n, ip, jp, 20, 20]
    #  L2 -> plain [128, OB, n*19*19]
    #  L3 -> parity padded 21x21 (ragged) as [128, n, ip, jp, 11, 12]
    #  L4 -> plain [128, OB, n*10*10]
    #  L5 -> parity 10x10 (no pad) as [128, n, ip, jp, 5, 6]
    # final -> fp32 out
    cfg = []
    for li, (Cin, Cout, k, stride, pad, Hi, Wi, Ho, Wo) in enumerate(shapes):
        CB = (Cin + 127) // 128
        OB = (Cout + 127) // 128
        KB = CB * k * k
        M = N_LOC * Ho * Wo
        cfg.append(
            dict(
                li=li, Cin=Cin, Cout=Cout, k=k, stride=stride, pad=pad,
                Hi=Hi, Wi=Wi, Ho=Ho, Wo=Wo, CB=CB, OB=OB, KB=KB, M=M,
                NM=n_total * Ho * Wo,
            )
        )

    # ---------------- DRAM I/O ----------------
    L1 = cfg[0]
    xt_d = nc.dram_tensor("xt", [L1["CB"], 128, N_LOC * L1["Hi"] * L1["Wi"]], BF16,
                          kind="ExternalInput")
    w_d, g_d, b_d = [], [], []
    for L in cfg:
        li = L["li"] + 1
        w_d.append(nc.dram_tensor(f"w{li}t", [L["KB"], 128, L["Cout"]], F32,
                                  kind="ExternalInput"))
        g_d.append(nc.dram_tensor(f"g{li}", [128, L["OB"]], F32, kind="ExternalInput"))
        b_d.append(nc.dram_tensor(f"b{li}", [128, L["OB"]], F32, kind="ExternalInput"))
    LL = cfg[-1]
    out_d = nc.dram_tensor("out", [N_LOC, LL["Cout"], LL["Ho"], LL["Wo"]], F32,
                           kind="ExternalOutput")

    # ---------------- persistent SBUF ----------------
    def sb(name, cols, dt):
        return nc.alloc_sbuf_tensor(name, [128, cols], dt)

    xsb = sb("xsb", L1["CB"] * N_LOC * L1["Hi"] * L1["Wi"], BF16)

    # activation tensors between layers
    # parity-split geometry helpers: (rows_even_arr, cols) per parity array
    act = []
    act.append(sb("act1p", cfg[0]["OB"] * N_LOC * 2 * 2 * 20 * 20, BF16))
    act.append(sb("act2", cfg[1]["OB"] * N_LOC * 19 * 19, BF16))
    act.append(sb("act3p", N_LOC * 2 * 2 * 11 * 12, BF16))
    act.append(sb("act4", cfg[3]["OB"] * N_LOC * 10 * 10, BF16))
    act.append(sb("act5p", N_LOC * 2 * 2 * 5 * 6, BF16))
    act.append(sb("fin6", cfg[5]["OB"] * N_LOC * 4 * 4, F32))

    S_max = max(L["OB"] * L["M"] for L in cfg)
    S = sb("S", S_max, F32)

    w_sb = [sb(f"wsb{L['li']}", L["KB"] * L["Cout"], F32) for L in cfg]
    g_sb = [sb(f"gsb{L['li']}", L["OB"], F32) for L in cfg]
    b_sb = [sb(f"bsb{L['li']}", L["OB"], F32) for L in cfg]

    ones = sb("ones", 1, BF16)
    # sliding one-hot window: OH[:, 31:32] = 1, rest 0.
    # lhsT for channel row q (0..31) = OH[:, 31-q : 63-q]  -> one-hot col q
    oh = sb("oh", 63, BF16)

    # per-layer stats and BN coefficient tensors (tiny)
    stats_sb, stats_g, abwork = {}, {}, {}
    sqfull = sb("sqfull", max(L["M"] for L in cfg), F32)

    st_in_d, st_out_d = {}, {}

    with tile.TileContext(nc) as tc:
        # ---------------- input loads ----------------
        for cb in range(L1["CB"]):
            W0 = N_LOC * L1["Hi"] * L1["Wi"]
            nc.sync.dma_start(xsb.ap()[:, cb * W0:(cb + 1) * W0], xt_d.ap()[cb])
        for L in cfg:
            li = L["li"]
            nc.sync.dma_start(
                w_sb[li].ap().rearrange("p (k c) -> p k c", k=L["KB"]),
                w_d[li].ap().rearrange("k p c -> p k c"),
            )
            nc.sync.dma_start(g_sb[li].ap(), g_d[li].ap())
            nc.sync.dma_start(b_sb[li].ap(), b_d[li].ap())
        nc.vector.memset(ones.ap(), 1.0)
        nc.vector.memset(oh.ap(), 0.0)
        nc.vector.memset(oh.ap()[:, 31:32], 1.0)
        # zero parity-padded act tensors (borders / pad columns stay zero)
        nc.gpsimd.memset(act[0].ap(), 0.0)
        nc.gpsimd.memset(act[2].ap(), 0.0)
        nc.gpsimd.memset(act[4].ap(), 0.0)

        with (
            tc.tile_pool(name="scr", bufs=8) as scrp,
            tc.tile_pool(name="ps", bufs=8, space="PSUM") as psp,
            tc.tile_pool(name="dram", bufs=2, space="DRAM") as dramp,
        ):
            # ============ patch AP generators ============
            # view helpers over activation tensors
            def patch_and_rhs(L):
                """returns (patch_fn(kb, ph) -> in0 AP matching scratch layout,
                            scr_len, phases, chunk list [(rhs_fn(scr), N, scol)])"""
                li = L["li"]
                k = L["k"]
                if li == 0:
                    W0 = N_LOC * L["Hi"] * L["Wi"]
                    def patch(kb, ph):
                        return xsb.ap()[:, kb * W0:(kb + 1) * W0]
                    cN = W0 // 8
                    chunks = [
                        (lambda scr, c=c: scr[:, c * cN:(c + 1) * cN], cN)
                        for c in range(8)
                    ]
                    return patch, W0, 1, chunks
                if k == 1:
                    src = act[li - 1]
                    Mfull = L["M"]  # n*Ho*Wo  (contiguous per cb)
                    def patch(kb, ph):
                        return src.ap()[:, kb * Mfull:(kb + 1) * Mfull]
                    nchunk = 2 if Mfull > 512 else 1
                    cN = Mfull // nchunk
                    chunks = [
                        (lambda scr, c=c: scr[:, c * cN:(c + 1) * cN], cN)
                        for c in range(nchunk)
                    ]
                    return patch, Mfull, 1, chunks
                # 3x3 layers: read parity-split source arrays
                if li == 1:
                    src, CBv, R, C, rows = act[0], L["CB"], 20, 20, 19
                elif li == 3:
                    src, CBv, R, C, rows = act[2], 1, 11, 12, 10
                else:
                    src, CBv, R, C, rows = act[4], 1, 5, 6, 4
                Ho, Wo = L["Ho"], L["Wo"]
                pad = L["pad"]

                if CBv > 1:
                    v = src.ap().rearrange(
                        "p (cb n ip jp r c) -> p cb n ip jp r c",
                        cb=CBv, n=N_LOC, ip=2, jp=2, r=R, c=C)
                else:
                    v = src.ap().rearrange(
                        "p (n ip jp r c) -> p n ip jp r c",
                        n=N_LOC, ip=2, jp=2, r=R, c=C)

                def sel(d):
                    # returns (parity_array, start_offset, rhs_col_offset)
                    if pad == 1:
                        return (1, 0, 0) if d == 0 else ((0, 0, 0) if d == 1 else (1, 1, 1))
                    return (0, 0, 0) if d == 0 else ((1, 0, 0) if d == 1 else (0, 1, 1))

                def patch(kb, ph):
                    cb, r9 = divmod(kb, 9)
                    dy, dx = divmod(r9, 3)
                    ipv, rs, _ = sel(dy)
                    jpv, _cs0, _ = sel(dx)
                    # full padded-width rows for alignment; rhs skips columns
                    if CBv > 1:
                        return v[:, cb, :, ipv, jpv, rs:rs + Ho, 0:C]
                    return v[:, :, ipv, jpv, rs:rs + Ho, 0:C]

                def rhs_cs(kb):
                    _, r9 = divmod(kb, 9)
                    _dy, dx = divmod(r9, 3)
                    return sel(dx)[2]

                scr_len = N_LOC * Ho * C
                if li == 1:
                    # one chunk per image: [19,19] of the [19,20] rows
                    def mk(nn):
                        def f(scr, kb):
                            cs = rhs_cs(kb)
                            sv = scr.rearrange("p (n r c) -> p n r c",
                                               n=N_LOC, r=Ho, c=C)
                            return sv[:, nn, :, cs:cs + Wo]
                        return f
                    chunks = [(mk(nn), Ho * Wo) for nn in range(N_LOC)]
                else:
                    def f(scr, kb):
                        cs = rhs_cs(kb)
                        sv = scr.rearrange("p (n r c) -> p n r c",
                                           n=N_LOC, r=Ho, c=C)
                        return sv[:, :, :, cs:cs + Wo]
                    chunks = [(f, N_LOC * Ho * Wo)]
                return patch, scr_len, 1, chunks

            # ============ conv layers ============
            ACT_FRAC = 0.385  # fraction of abs-diff tiles on ScalarE
            eng_acc = [0.0]
            for L in cfg:
                li, Cout, KB, M, OB = L["li"], L["Cout"], L["KB"], L["M"], L["OB"]
                patch, scr_len, phases, chunks = patch_and_rhs(L)
                nchunk = len(chunks)
                stats_sb[li] = sb(f"stats{li}", 2 * OB, F32)
                stats_g[li] = sb(f"statsg{li}", 2 * OB, F32)
                abwork[li] = sb(f"abw{li}", 12 * OB, F32)

                Ho, Wo = L["Ho"], L["Wo"]

                def finish_ob(ob, li=li, L=L, M=M, OB=OB, Ho=Ho, Wo=Wo):
                    # per-block stats -> AllReduce -> BN coeffs -> epilogue;
                    # runs concurrently with the next block's conv on PE
                    nc.vector.tensor_scalar(
                        sqfull.ap()[:, :M], S.ap()[:, ob * M:(ob + 1) * M],
                        0.0, 0.0, op0=OP.add, op1=OP.add,
                        accum_out=stats_sb[li].ap()[:, 2 * ob:2 * ob + 1],
                    )
                    nc.scalar.activation(
                        sqfull.ap()[:, :M], S.ap()[:, ob * M:(ob + 1) * M],
                        AF.Square,
                        accum_out=stats_sb[li].ap()[:, 2 * ob + 1:2 * ob + 2],
                    )
                    sti = dramp.tile([128, 2], F32, tag=f"sti{li}_{ob}",
                                     name=f"sti{li}_{ob}")
                    sto = dramp.tile([128, 2], F32, tag=f"sto{li}_{ob}",
                                     name=f"sto{li}_{ob}")
                    nc.sync.dma_start(sti[:, :],
                                      stats_sb[li].ap()[:, 2 * ob:2 * ob + 2])
                    if mock_cc:
                        nc.sync.dma_start(sto[:, :], sti[:, :])
                    else:
                        nc.gpsimd.collective_compute(
                            "AllReduce", OP.add,
                            replica_groups=[list(range(n_cores))],
                            ins=[sti.opt()], outs=[sto.opt()],
                        )
                    nc.sync.dma_start(stats_g[li].ap()[:, 2 * ob:2 * ob + 2],
                                      sto[:, :])

                    aw = abwork[li].ap()

                    def col(i):
                        return aw[:, i * OB + ob:i * OB + ob + 1]

                    (mean, ex2, m2, vpe, sq_, rc, u, s2, rinv, ga, negA,
                     Bv) = [col(i) for i in range(12)]
                    inm = 1.0 / L["NM"]
                    nc.vector.tensor_scalar_mul(
                        mean, stats_g[li].ap()[:, 2 * ob:2 * ob + 1], inm)
                    nc.vector.tensor_scalar_mul(
                        ex2, stats_g[li].ap()[:, 2 * ob + 1:2 * ob + 2], inm)
                    nc.scalar.activation(m2, mean, AF.Square)
                    nc.vector.tensor_tensor(vpe, ex2, m2, op=OP.subtract)
                    nc.vector.tensor_scalar_add(vpe, vpe, EPS)
                    nc.scalar.activation(sq_, vpe, AF.Sqrt)
                    nc.vector.reciprocal(rc, sq_)
                    nc.vector.tensor_tensor(u, vpe, rc, op=OP.mult)
                    nc.vector.tensor_tensor(s2, sq_, u, op=OP.add)
                    nc.vector.tensor_scalar_mul(s2, s2, 0.5)
                    nc.vector.reciprocal(rinv, s2)
                    gcol = g_sb[li].ap()[:, ob:ob + 1]
                    bcol = b_sb[li].ap()[:, ob:ob + 1]
                    nc.vector.tensor_tensor(ga, gcol, rinv, op=OP.mult)
                    nc.vector.tensor_scalar_mul(negA, ga, -1.0)
                    nc.vector.tensor_tensor(Bv, mean, ga, op=OP.mult)
                    nc.vector.tensor_tensor(Bv, bcol, Bv, op=OP.add)

                    def apply_tsp(dst_ap, src_ap):
                        nc.vector.tensor_scalar(
                            dst_ap, src_ap, negA, Bv, OP.mult, OP.add)

                    if li in (0, 2, 4):
                        if li == 0:
                            R, C = 20, 20
                            dstv = act[0].ap().rearrange(
                                "p (obb n ip jp r c) -> p obb n ip jp r c",
                                obb=OB, n=N_LOC, ip=2, jp=2, r=R, c=C)
                        elif li == 2:
                            R, C = 11, 12
                            dstv = act[2].ap().rearrange(
                                "p (n ip jp r c) -> p n ip jp r c",
                                n=N_LOC, ip=2, jp=2, r=R, c=C)
                        else:
                            R, C = 5, 6
                            dstv = act[4].ap().rearrange(
                                "p (n ip jp r c) -> p n ip jp r c",
                                n=N_LOC, ip=2, jp=2, r=R, c=C)
                        Sv = S.ap()[:, :OB * M].rearrange(
                            "p (obb n i j) -> p obb n i j",
                            obb=OB, n=N_LOC, i=Ho, j=Wo)
                        pad = 1 if li in (0, 2) else 0
                        for bpar in (0, 1):
                            for dpar in (0, 1):
                                srcv = Sv[:, ob, :, bpar::2, dpar::2]
                                na, ncc = srcv.shape[2], srcv.shape[3]
                                if pad == 1:
                                    ipv, rs = (0, 0) if bpar == 0 else (1, 1)
                                    jpv, cs = (0, 0) if dpar == 0 else (1, 1)
                                else:
                                    ipv, rs = (0, 0) if bpar == 0 else (1, 0)
                                    jpv, cs = (0, 0) if dpar == 0 else (1, 0)
                                if li == 0:
                                    dst = dstv[:, ob, :, ipv, jpv,
                                               rs:rs + na, cs:cs + ncc]
                                else:
                                    dst = dstv[:, :, ipv, jpv,
                                               rs:rs + na, cs:cs + ncc]
                                apply_tsp(dst, srcv)
                        blk = act[li].ap().shape[1] // OB
                        tgt = act[li].ap()[:, ob * blk:(ob + 1) * blk]
                        nc.vector.tensor_scalar(tgt, tgt, 0.0, 6.0,
                                                OP.max, OP.min)
                    else:
                        dst_t = act[li] if li != 5 else act[5]
                        apply_tsp(dst_t.ap()[:, ob * M:(ob + 1) * M],
                                  S.ap()[:, ob * M:(ob + 1) * M])
                        tgt = dst_t.ap()[:, ob * M:(ob + 1) * M]
                        nc.vector.tensor_scalar(tgt, tgt, 0.0, 6.0,
                                                OP.max, OP.min)

                for ob in range(OB):
                    pss = [
                        psp.tile([128, chunks[c][1]], F32, tag="ps",
                                 name=f"ps{li}_{c}",
                                 padded_shape=[128, 512])
                        for c in range(nchunk)
                    ]
                    for kb in range(KB):
                        for o_r in range(128):
                            o = ob * 128 + o_r
                            quad, q = divmod(o_r, 32)
                            scr = scrp.tile([128, scr_len], BF16, tag="scr")
                            wap = w_sb[li].ap()[:, kb * Cout + o:kb * Cout + o + 1]
                            eng_acc[0] += ACT_FRAC
                            if eng_acc[0] >= 1.0:
                                eng_acc[0] -= 1.0
                                nc.scalar.activation(
                                    scr[:, :scr_len], patch(kb, 0), AF.Abs,
                                    bias=wap, scale=1.0)
                            else:
                                nc.vector.tensor_scalar_add(
                                    scr[:, :scr_len], patch(kb, 0), wap)
                                u = scr[:, :scr_len].bitcast(U16)
                                nc.vector.tensor_scalar(
                                    u, u, 0x7FFF, None,
                                    OP.bitwise_and, OP.bypass)
                            for c in range(nchunk):
                                rhs_fn, cN = chunks[c]
                                rhs = (rhs_fn(scr, kb) if L["k"] == 3
                                       else rhs_fn(scr))
                                nc.tensor.matmul(
                                    pss[c][32 * quad:32 * quad + 32, :],
                                    oh.ap()[:, 31 - q:63 - q], rhs,
                                    start=(q == 0 and kb == 0),
                                    stop=(q == 31 and kb == KB - 1),
                                    tile_position=(0, 32 * quad),
                                    skip_group_check=True,
                                )
                    # evacuate whole 128-channel block -> S (aligned)
                    for c in range(nchunk):
                        cN = chunks[c][1]
                        scol = ob * M + c * cN
                        nc.vector.tensor_scalar_add(
                            S.ap()[:, scol:scol + cN], pss[c][:, :cN], 0.0,
                        )
                    finish_ob(ob)

            # ---------------- final output DMA ----------------
            hw = LL["Ho"] * LL["Wo"]
            finv = act[5].ap().rearrange("p (ob n hw) -> p ob n hw",
                                         ob=LL["OB"], n=N_LOC, hw=hw)
            dst = out_d.ap().rearrange("n (ob p) h w -> p ob n (h w)",
                                       ob=LL["OB"], p=128)
            for ob in range(LL["OB"]):
                nc.sync.dma_start(dst[:, ob], finv[:, ob])

    nc.compile()
    return nc


def _get_nc():
    if "nc" not in _NC_CACHE:
        _NC_CACHE["nc"] = _build_nc()
    return _NC_CACHE["nc"]


def kernel(**inputs) -> np.ndarray:
    import time as _time
    from concourse.bass_utils import run_bass_kernel_spmd

    nc = _get_nc()
    in_maps = _host_prep(inputs)
    last = None
    for attempt in range(3):
        try:
            res = run_bass_kernel_spmd(nc, in_maps, core_ids=list(range(N_CORES)))
            outs = [np.asarray(r["out"]).reshape(N_LOC, 256, 4, 4)
                    for r in res.results]
            return np.concatenate(outs, axis=0).astype(np.float32)
        except Exception as e:  # transient axon mesh desync: wait + retry
            last = e
            _time.sleep(20 * (attempt + 1))
    raise last


if __name__ == "__main__":
    nc = _build_nc()
    print("build + compile OK")



# revision 24
# speedup vs baseline: 1.3968x; 1.3968x over previous
"""AdderNet CNN (6x adder_conv + sync-BN + ReLU6) on 8 Trainium2 NeuronCores.

Sharding: data-parallel over batch (2 images/core), sync-BN via ONE
AllReduce of per-channel (sum, sumsq) per layer (6 total; per-block
collectives serialize the pipeline and cost ~20ms extra wall).

Per-layer compute (per core):
  - contraction dim k lives on SBUF partitions (128-sized k-blocks)
  - abs-diff feed split ~38/62 between ScalarE (activation Abs with
    per-partition bias = -w) and VectorE (tensor_scalar add + uint16
    bitwise_and 0x7FFF sign-clear), both writing bf16 scratch tiles
  - k-partition-sum via TensorE matmul with a sliding one-hot [128,32]
    stationary matrix: channel o accumulates into PSUM row o%128 of a
    shared per-chunk tile (PE array tiling tile_position=(0,32*quad);
    first matmul of each 32-row quadrant resets it via start=True), so a
    full 128-channel block evacuates with one aligned [128,N] DVE op
  - per-channel sum/sumsq via tensor_scalar/activation(Square) accum_out,
    AllReduce of stats (sync-BN), then BN+ReLU6 via tensor_scalar
    (mult,add then max,min), writing bf16 activations in parity-split
    padded layouts so stride-2 consumers read unit-stride, even-length,
    4B-aligned rows (keeps DVE 4x / clean PE rhs APs).
"""

import sys
import numpy as np

if "/opt/trn_rl_repo" not in sys.path:
    sys.path.insert(0, "/opt/trn_rl_repo")

import ml_dtypes

N_CORES = 8
N_LOC = 2  # images per core
EPS = 1e-5

# (Cin, Cout, k, stride, pad, Hi, Wi, Ho, Wo)
LAYER_SHAPES = [
    (512, 256, 1, 1, 0, 38, 38, 38, 38),
    (256, 512, 3, 2, 1, 38, 38, 19, 19),
    (512, 128, 1, 1, 0, 19, 19, 19, 19),
    (128, 256, 3, 2, 1, 19, 19, 10, 10),
    (256, 128, 1, 1, 0, 10, 10, 10, 10),
    (128, 256, 3, 2, 0, 10, 10, 4, 4),
]

_NC_CACHE = {}


def _host_prep(inputs, shapes=LAYER_SHAPES, n_total=16):
    """Build per-core in_maps from the raw reference inputs."""
    bf16 = ml_dtypes.bfloat16
    x = np.asarray(inputs["x"], dtype=np.float32)
    maps_common = {}
    for li, (Cin, Cout, k, *_rest) in enumerate(shapes, start=1):
        w = np.asarray(inputs[f"w{li}"], dtype=np.float32)  # [Cout,Cin,k,k]
        CB = (Cin + 127) // 128
        if k == 1:
            wt = w[:, :, 0, 0].T.reshape(CB, 128, Cout)
        else:
            # kb = cb*9 + dy*3 + dx
            wt = (
                w.transpose(1, 2, 3, 0)  # [Cin, k, k, Cout]
                .reshape(CB, 128, k * k, Cout)
                .transpose(0, 2, 1, 3)  # [CB, k*k, 128, Cout]
                .reshape(CB * k * k, 128, Cout)
            )
        # negated: device computes |x + w'| with w' = -w
        maps_common[f"w{li}t"] = np.ascontiguousarray(-wt, dtype=np.float32)
        OB = (Cout + 127) // 128
        g = np.asarray(inputs[f"g{li}"], dtype=np.float32).reshape(OB, 128).T
        b = np.asarray(inputs[f"b{li}"], dtype=np.float32).reshape(OB, 128).T
        maps_common[f"g{li}"] = np.ascontiguousarray(g)
        maps_common[f"b{li}"] = np.ascontiguousarray(b)

    Cin0, _, _, _, _, Hi, Wi, _, _ = shapes[0]
    CB0 = (Cin0 + 127) // 128
    in_maps = []
    n_cores = n_total // N_LOC
    for c in range(n_cores):
        xs = x[c * N_LOC : (c + 1) * N_LOC]  # [2, Cin, H, W]
        xt = (
            xs.transpose(1, 0, 2, 3)
            .reshape(CB0, 128, N_LOC * Hi * Wi)
            .astype(bf16)
        )
        m = dict(maps_common)
        m["xt"] = np.ascontiguousarray(xt)
        in_maps.append(m)
    return in_maps


def _build_nc(shapes=LAYER_SHAPES, n_total=16, n_cores=N_CORES, mock_cc=False):
    import concourse.bacc as bacc
    import concourse.mybir as mybir
    from concourse import tile

    OP = mybir.AluOpType
    AF = mybir.ActivationFunctionType
    BF16 = mybir.dt.bfloat16
    F32 = mybir.dt.float32
    U16 = mybir.dt.uint16

    nc = bacc.Bacc(
        "TRN2",
        target_bir_lowering=False,
        debug=False,
        enable_asserts=False,
        num_devices=n_cores,
    )

    # ---------------- derived per-layer geometry ----------------
    # Activation storage layouts (per layer output), bf16:
    #  L1 -> parity-split padded 40x40 grid: [128, OB, n, ip, jp, 20, 20]
    #  L2 -> plain [128, OB, n*19*19]
    #  L3 -> parity padded 21x21 (ragged) as [128, n, ip, jp, 11, 12]
    #  L4 -> plain [128, OB, n*10*10]
    #  L5 -> parity 10x10 (no pad) as [128, n, ip, jp, 5, 6]
    # final -> fp32 out
    cfg = []
    for li, (Cin, Cout, k, stride, pad, Hi, Wi, Ho, Wo) in enumerate(shapes):
        CB = (Cin + 127) // 128
        OB = (Cout + 127) // 128
        KB = CB * k * k
        nl = N_LOC
        M = nl * Ho * Wo
        cfg.append(
            dict(
                li=li, Cin=Cin, Cout=Cout, k=k, stride=stride, pad=pad,
                Hi=Hi, Wi=Wi, Ho=Ho, Wo=Wo, CB=CB, OB=OB, KB=KB, M=M,
                nl=nl, NM=n_total * Ho * Wo,
            )
        )

    # ---------------- DRAM I/O ----------------
    L1 = cfg[0]
    xt_d = nc.dram_tensor("xt", [L1["CB"], 128, N_LOC * L1["Hi"] * L1["Wi"]], BF16,
                          kind="ExternalInput")
    w_d, g_d, b_d = [], [], []
    for L in cfg:
        li = L["li"] + 1
        w_d.append(nc.dram_tensor(f"w{li}t", [L["KB"], 128, L["Cout"]], F32,
                                  kind="ExternalInput"))
        g_d.append(nc.dram_tensor(f"g{li}", [128, L["OB"]], F32, kind="ExternalInput"))
        b_d.append(nc.dram_tensor(f"b{li}", [128, L["OB"]], F32, kind="ExternalInput"))
    LL = cfg[-1]
    out_d = nc.dram_tensor("out", [N_LOC, LL["Cout"], LL["Ho"], LL["Wo"]], F32,
                           kind="ExternalOutput")

    # ---------------- persistent SBUF ----------------
    def sb(name, cols, dt):
        return nc.alloc_sbuf_tensor(name, [128, cols], dt)

    xsb = sb("xsb", L1["CB"] * N_LOC * L1["Hi"] * L1["Wi"], BF16)

    # activation tensors between layers
    # parity-split geometry helpers: (rows_even_arr, cols) per parity array
    act = []
    act.append(sb("act1p", cfg[0]["OB"] * N_LOC * 2 * 2 * 20 * 20, BF16))
    act.append(sb("act2", cfg[1]["OB"] * N_LOC * 19 * 19, BF16))
    act.append(sb("act3p", N_LOC * 2 * 2 * 11 * 12, BF16))
    act.append(sb("act4", cfg[3]["OB"] * N_LOC * 10 * 10, BF16))
    act.append(sb("act5p", N_LOC * 2 * 2 * 5 * 6, BF16))
    act.append(sb("fin6", cfg[5]["OB"] * N_LOC * 4 * 4, F32))

    S_max = max(L["OB"] * L["M"] for L in cfg)
    S = sb("S", S_max, F32)

    w_sb = [sb(f"wsb{L['li']}", L["KB"] * L["Cout"], F32) for L in cfg]
    g_sb = [sb(f"gsb{L['li']}", L["OB"], F32) for L in cfg]
    b_sb = [sb(f"bsb{L['li']}", L["OB"], F32) for L in cfg]

    ones = sb("ones", 1, BF16)
    # sliding one-hot window: OH[:, 31:32] = 1, rest 0.
    # lhsT for channel row q (0..31) = OH[:, 31-q : 63-q]  -> one-hot col q
    oh = sb("oh", 63, BF16)

    # per-layer stats and BN coefficient tensors (tiny)
    stats_sb, stats_g, abwork = {}, {}, {}
    sqfull = sb("sqfull", max(L["M"] for L in cfg), F32)

    st_in_d, st_out_d = {}, {}

    with tile.TileContext(nc) as tc:
        # ---------------- input loads ----------------
        for cb in range(L1["CB"]):
            W0 = N_LOC * L1["Hi"] * L1["Wi"]
            nc.sync.dma_start(xsb.ap()[:, cb * W0:(cb + 1) * W0], xt_d.ap()[cb])
        for L in cfg:
            li = L["li"]
            nc.sync.dma_start(
                w_sb[li].ap().rearrange("p (k c) -> p k c", k=L["KB"]),
                w_d[li].ap().rearrange("k p c -> p k c"),
            )
            nc.sync.dma_start(g_sb[li].ap(), g_d[li].ap())
            nc.sync.dma_start(b_sb[li].ap(), b_d[li].ap())
        nc.vector.memset(ones.ap(), 1.0)
        nc.vector.memset(oh.ap(), 0.0)
        nc.vector.memset(oh.ap()[:, 31:32], 1.0)
        # zero parity-padded act tensors (borders / pad columns stay zero)
        nc.gpsimd.memset(act[0].ap(), 0.0)
        nc.gpsimd.memset(act[2].ap(), 0.0)
        nc.gpsimd.memset(act[4].ap(), 0.0)

        with (
            tc.tile_pool(name="scr", bufs=8) as scrp,
            tc.tile_pool(name="ps", bufs=8, space="PSUM") as psp,
            tc.tile_pool(name="dram", bufs=2, space="DRAM") as dramp,
        ):
            # ============ patch AP generators ============
            # view helpers over activation tensors
            def patch_and_rhs(L):
                """returns (patch_fn(kb, ph) -> in0 AP matching scratch layout,
                            scr_len, phases, chunk list [(rhs_fn(scr), N, scol)])"""
                li = L["li"]
                k = L["k"]
                if li == 0:
                    W0 = N_LOC * L["Hi"] * L["Wi"]
                    def patch(kb, ph):
                        return xsb.ap()[:, kb * W0:(kb + 1) * W0]
                    # 6 chunks: 5x512 + 328 (PSUM bank limit 512 fp32 cols)
                    bounds = [0, 512, 1024, 1536, 2048, 2560, W0]
                    chunks = [
                        (lambda scr, c=c: scr[:, bounds[c]:bounds[c + 1]],
                         bounds[c + 1] - bounds[c])
                        for c in range(6)
                    ]
                    return patch, W0, 1, chunks
                if k == 1:
                    src = act[li - 1]
                    Mfull = L["M"]  # n*Ho*Wo  (contiguous per cb)
                    def patch(kb, ph):
                        return src.ap()[:, kb * Mfull:(kb + 1) * Mfull]
                    nchunk = max(1, -(-Mfull // 512))
                    while Mfull % nchunk:
                        nchunk += 1
                    cN = Mfull // nchunk
                    chunks = [
                        (lambda scr, c=c: scr[:, c * cN:(c + 1) * cN], cN)
                        for c in range(nchunk)
                    ]
                    return patch, Mfull, 1, chunks
                # 3x3 layers: read parity-split source arrays
                if li == 1:
                    src, CBv, R, C, rows = act[0], L["CB"], 20, 20, 19
                elif li == 3:
                    src, CBv, R, C, rows = act[2], 1, 11, 12, 10
                else:
                    src, CBv, R, C, rows = act[4], 1, 5, 6, 4
                Ho, Wo = L["Ho"], L["Wo"]
                pad = L["pad"]
                nl = L["nl"]

                if CBv > 1:
                    v = src.ap().rearrange(
                        "p (cb n ip jp r c) -> p cb n ip jp r c",
                        cb=CBv, n=nl, ip=2, jp=2, r=R, c=C)
                else:
                    v = src.ap().rearrange(
                        "p (n ip jp r c) -> p n ip jp r c",
                        n=nl, ip=2, jp=2, r=R, c=C)

                def sel(d):
                    # returns (parity_array, start_offset, rhs_col_offset)
                    if pad == 1:
                        return (1, 0, 0) if d == 0 else ((0, 0, 0) if d == 1 else (1, 1, 1))
                    return (0, 0, 0) if d == 0 else ((1, 0, 0) if d == 1 else (0, 1, 1))

                def patch(kb, ph):
                    cb, r9 = divmod(kb, 9)
                    dy, dx = divmod(r9, 3)
                    ipv, rs, _ = sel(dy)
                    jpv, _cs0, _ = sel(dx)
                    # full padded-width rows for alignment; rhs skips columns
                    if CBv > 1:
                        return v[:, cb, :, ipv, jpv, rs:rs + Ho, 0:C]
                    return v[:, :, ipv, jpv, rs:rs + Ho, 0:C]

                def rhs_cs(kb):
                    _, r9 = divmod(kb, 9)
                    _dy, dx = divmod(r9, 3)
                    return sel(dx)[2]

                scr_len = nl * Ho * C
                if li == 1:
                    # one chunk per image: [19,19] of the [19,20] rows
                    def mk(nn):
                        def f(scr, kb):
                            cs = rhs_cs(kb)
                            sv = scr.rearrange("p (n r c) -> p n r c",
                                               n=nl, r=Ho, c=C)
                            return sv[:, nn, :, cs:cs + Wo]
                        return f
                    chunks = [(mk(nn), Ho * Wo) for nn in range(nl)]
                else:
                    def f(scr, kb):
                        cs = rhs_cs(kb)
                        sv = scr.rearrange("p (n r c) -> p n r c",
                                           n=nl, r=Ho, c=C)
                        return sv[:, :, :, cs:cs + Wo]
                    chunks = [(f, nl * Ho * Wo)]
                return patch, scr_len, 1, chunks

            # ============ conv layers ============
            ACT_FRAC = 0.45  # fraction of abs-diff tiles on ScalarE
            eng_acc = [0.0]
            for L in cfg:
                li, Cout, KB, M, OB = L["li"], L["Cout"], L["KB"], L["M"], L["OB"]
                patch, scr_len, phases, chunks = patch_and_rhs(L)
                nchunk = len(chunks)
                stats_sb[li] = sb(f"stats{li}", 2 * OB, F32)
                stats_g[li] = sb(f"statsg{li}", 2 * OB, F32)
                abwork[li] = sb(f"abw{li}", 12 * OB, F32)

                Ho, Wo = L["Ho"], L["Wo"]

                def stats_ob(ob, li=li, M=M):
                    # per-block stats accumulation; overlaps next block's conv
                    nc.vector.tensor_scalar(
                        sqfull.ap()[:, :M], S.ap()[:, ob * M:(ob + 1) * M],
                        0.0, 0.0, op0=OP.add, op1=OP.add,
                        accum_out=stats_sb[li].ap()[:, 2 * ob:2 * ob + 1],
                    )
                    nc.scalar.activation(
                        sqfull.ap()[:, :M], S.ap()[:, ob * M:(ob + 1) * M],
                        AF.Square,
                        accum_out=stats_sb[li].ap()[:, 2 * ob + 1:2 * ob + 2],
                    )

                def layer_allreduce(li=li, OB=OB):
                    # one AllReduce of all blocks' (sum, sumsq) for this layer
                    sti = dramp.tile([128, 2 * OB], F32, tag=f"sti{li}",
                                     name=f"sti{li}")
                    sto = dramp.tile([128, 2 * OB], F32, tag=f"sto{li}",
                                     name=f"sto{li}")
                    nc.sync.dma_start(sti[:, :], stats_sb[li].ap())
                    if mock_cc:
                        nc.sync.dma_start(sto[:, :], sti[:, :])
                    else:
                        nc.gpsimd.collective_compute(
                            "AllReduce", OP.add,
                            replica_groups=[list(range(n_cores))],
                            ins=[sti.opt()], outs=[sto.opt()],
                        )
                    nc.sync.dma_start(stats_g[li].ap(), sto[:, :])

                def finish_ob(ob, li=li, L=L, M=M, OB=OB, Ho=Ho, Wo=Wo):
                    # BN coeffs from global stats -> epilogue
                    aw = abwork[li].ap()

                    def col(i):
                        return aw[:, i * OB + ob:i * OB + ob + 1]

                    (mean, ex2, m2, vpe, sq_, rc, u, s2, rinv, ga, negA,
                     Bv) = [col(i) for i in range(12)]
                    inm = 1.0 / L["NM"]
                    nc.vector.tensor_scalar_mul(
                        mean, stats_g[li].ap()[:, 2 * ob:2 * ob + 1], inm)
                    nc.vector.tensor_scalar_mul(
                        ex2, stats_g[li].ap()[:, 2 * ob + 1:2 * ob + 2], inm)
                    nc.scalar.activation(m2, mean, AF.Square)
                    nc.vector.tensor_tensor(vpe, ex2, m2, op=OP.subtract)
                    nc.vector.tensor_scalar_add(vpe, vpe, EPS)
                    nc.scalar.activation(sq_, vpe, AF.Sqrt)
                    nc.vector.reciprocal(rc, sq_)
                    nc.vector.tensor_tensor(u, vpe, rc, op=OP.mult)
                    nc.vector.tensor_tensor(s2, sq_, u, op=OP.add)
                    nc.vector.tensor_scalar_mul(s2, s2, 0.5)
                    nc.vector.reciprocal(rinv, s2)
                    gcol = g_sb[li].ap()[:, ob:ob + 1]
                    bcol = b_sb[li].ap()[:, ob:ob + 1]
                    nc.vector.tensor_tensor(ga, gcol, rinv, op=OP.mult)
                    nc.vector.tensor_scalar_mul(negA, ga, -1.0)
                    nc.vector.tensor_tensor(Bv, mean, ga, op=OP.mult)
                    nc.vector.tensor_tensor(Bv, bcol, Bv, op=OP.add)

                    def apply_tsp(dst_ap, src_ap):
                        nc.vector.tensor_scalar(
                            dst_ap, src_ap, negA, Bv, OP.mult, OP.add)

                    if li in (0, 2, 4):
                        if li == 0:
                            R, C = 20, 20
                            dstv = act[0].ap().rearrange(
                                "p (obb n ip jp r c) -> p obb n ip jp r c",
                                obb=OB, n=N_LOC, ip=2, jp=2, r=R, c=C)
                        elif li == 2:
                            R, C = 11, 12
                            dstv = act[2].ap().rearrange(
                                "p (n ip jp r c) -> p n ip jp r c",
                                n=N_LOC, ip=2, jp=2, r=R, c=C)
                        else:
                            R, C = 5, 6
                            dstv = act[4].ap().rearrange(
                                "p (n ip jp r c) -> p n ip jp r c",
                                n=N_LOC, ip=2, jp=2, r=R, c=C)
                        Sv = S.ap()[:, :OB * M].rearrange(
                            "p (obb n i j) -> p obb n i j",
                            obb=OB, n=N_LOC, i=Ho, j=Wo)
                        pad = 1 if li in (0, 2) else 0
                        for bpar in (0, 1):
                            for dpar in (0, 1):
                                srcv = Sv[:, ob, :, bpar::2, dpar::2]
                                na, ncc = srcv.shape[2], srcv.shape[3]
                                if pad == 1:
                                    ipv, rs = (0, 0) if bpar == 0 else (1, 1)
                                    jpv, cs = (0, 0) if dpar == 0 else (1, 1)
                                else:
                                    ipv, rs = (0, 0) if bpar == 0 else (1, 0)
                                    jpv, cs = (0, 0) if dpar == 0 else (1, 0)
                                if li == 0:
                                    dst = dstv[:, ob, :, ipv, jpv,
                                               rs:rs + na, cs:cs + ncc]
                                else:
                                    dst = dstv[:, :, ipv, jpv,
                                               rs:rs + na, cs:cs + ncc]
                                apply_tsp(dst, srcv)
                        blk = act[li].ap().shape[1] // OB
                        tgt = act[li].ap()[:, ob * blk:(ob + 1) * blk]
                        nc.vector.tensor_scalar(tgt, tgt, 0.0, 6.0,
                                                OP.max, OP.min)
                    else:
                        dst_t = act[li] if li != 5 else act[5]
                        apply_tsp(dst_t.ap()[:, ob * M:(ob + 1) * M],
                                  S.ap()[:, ob * M:(ob + 1) * M])
                        tgt = dst_t.ap()[:, ob * M:(ob + 1) * M]
                        nc.vector.tensor_scalar(tgt, tgt, 0.0, 6.0,
                                                OP.max, OP.min)

                for ob in range(OB):
                    pss = [
                        psp.tile([128, chunks[c][1]], F32, tag="ps",
                                 name=f"ps{li}_{c}",
                                 padded_shape=[128, 512])
                        for c in range(nchunk)
                    ]
                    for kb in range(KB):
                        for o_r in range(128):
                            o = ob * 128 + o_r
                            quad, q = divmod(o_r, 32)
                            scr = scrp.tile([128, scr_len], BF16, tag="scr")
                            wap = w_sb[li].ap()[:, kb * Cout + o:kb * Cout + o + 1]
                            eng_acc[0] += ACT_FRAC
                            if eng_acc[0] >= 1.0:
                                eng_acc[0] -= 1.0
                                nc.scalar.activation(
                                    scr[:, :scr_len], patch(kb, 0), AF.Abs,
                                    bias=wap, scale=1.0)
                            else:
                                nc.vector.tensor_scalar_add(
                                    scr[:, :scr_len], patch(kb, 0), wap)
                                u = scr[:, :scr_len].bitcast(U16)
                                nc.vector.tensor_scalar(
                                    u, u, 0x7FFF, None,
                                    OP.bitwise_and, OP.bypass)
                            for c in range(nchunk):
                                rhs_fn, cN = chunks[c]
                                rhs = (rhs_fn(scr, kb) if L["k"] == 3
                                       else rhs_fn(scr))
                                nc.tensor.matmul(
                                    pss[c][32 * quad:32 * quad + 32, :],
                                    oh.ap()[:, 31 - q:63 - q], rhs,
                                    start=(q == 0 and kb == 0),
                                    stop=(q == 31 and kb == KB - 1),
                                    tile_position=(0, 32 * quad),
                                    skip_group_check=True,
                                )
                    # evacuate whole 128-channel block -> S (aligned)
                    coff = 0
                    for c in range(nchunk):
                        cN = chunks[c][1]
                        scol = ob * M + coff
                        nc.vector.tensor_scalar_add(
                            S.ap()[:, scol:scol + cN], pss[c][:, :cN], 0.0,
                        )
                        coff += cN
                    stats_ob(ob)
                layer_allreduce()
                for ob in range(OB):
                    finish_ob(ob)

            # ---------------- final output DMA ----------------
            hw = LL["Ho"] * LL["Wo"]
            finv = act[5].ap().rearrange("p (ob n hw) -> p ob n hw",
                                         ob=LL["OB"], n=N_LOC, hw=hw)
            dst = out_d.ap().rearrange("n (ob p) h w -> p ob n (h w)",
                                       ob=LL["OB"], p=128)
            for ob in range(LL["OB"]):
                nc.sync.dma_start(dst[:, ob], finv[:, ob])

    nc.compile()
    return nc


def _get_nc():
    if "nc" not in _NC_CACHE:
        _NC_CACHE["nc"] = _build_nc()
    return _NC_CACHE["nc"]


def kernel(**inputs) -> np.ndarray:
    import time as _time
    from concourse.bass_utils import run_bass_kernel_spmd

    nc = _get_nc()
    in_maps = _host_prep(inputs)
    last = None
    for attempt in range(3):
        try:
            res = run_bass_kernel_spmd(nc, in_maps, core_ids=list(range(N_CORES)))
            outs = [np.asarray(r["out"]).reshape(N_LOC, 256, 4, 4)
                    for r in res.results]
            return np.concatenate(outs, axis=0).astype(np.float32)
        except Exception as e:  # transient axon mesh desync: wait + retry
            last = e
            _time.sleep(20 * (attempt + 1))
    raise last


if __name__ == "__main__":
    nc = _build_nc()
    print("build + compile OK")



# revision 25
# speedup vs baseline: 1.4033x; 1.0047x over previous
"""AdderNet CNN (6x adder_conv + sync-BN + ReLU6) on 8 Trainium2 NeuronCores.

Sharding: data-parallel over batch (2 images/core), sync-BN via ONE
AllReduce of per-channel (sum, sumsq) per layer (6 total; per-block
collectives serialize the pipeline and cost ~20ms extra wall).

Per-layer compute (per core):
  - contraction dim k lives on SBUF partitions (128-sized k-blocks)
  - abs-diff feed split ~45/55 between ScalarE (activation Abs with
    per-partition bias = -w, ~151 G elem/s) and VectorE (tensor_scalar
    add + uint16 bitwise_and 0x7FFF sign-clear, 2 passes at ~371 G
    elem-visits/s), both writing bf16 scratch tiles
  - k-partition-sum via TensorE matmul with a sliding one-hot [128,32]
    stationary matrix: channel o accumulates into PSUM row o%128 of a
    shared per-chunk tile (PE array tiling tile_position=(0,32*quad);
    first matmul of each 32-row quadrant resets it via start=True), so a
    full 128-channel block evacuates with one aligned [128,N] DVE op
  - per-channel sum/sumsq via tensor_scalar/activation(Square) accum_out,
    AllReduce of stats (sync-BN), then BN+ReLU6 via tensor_scalar
    (mult,add then max,min), writing bf16 activations in parity-split
    padded layouts so stride-2 consumers read unit-stride, even-length,
    4B-aligned rows (keeps DVE 4x / clean PE rhs APs).
"""

import sys
import numpy as np

if "/opt/trn_rl_repo" not in sys.path:
    sys.path.insert(0, "/opt/trn_rl_repo")

import ml_dtypes

N_CORES = 8
N_LOC = 2  # images per core
EPS = 1e-5

# (Cin, Cout, k, stride, pad, Hi, Wi, Ho, Wo)
LAYER_SHAPES = [
    (512, 256, 1, 1, 0, 38, 38, 38, 38),
    (256, 512, 3, 2, 1, 38, 38, 19, 19),
    (512, 128, 1, 1, 0, 19, 19, 19, 19),
    (128, 256, 3, 2, 1, 19, 19, 10, 10),
    (256, 128, 1, 1, 0, 10, 10, 10, 10),
    (128, 256, 3, 2, 0, 10, 10, 4, 4),
]

_NC_CACHE = {}


def _host_prep(inputs, shapes=LAYER_SHAPES, n_total=16):
    """Build per-core in_maps from the raw reference inputs."""
    bf16 = ml_dtypes.bfloat16
    x = np.asarray(inputs["x"], dtype=np.float32)
    maps_common = {}
    for li, (Cin, Cout, k, *_rest) in enumerate(shapes, start=1):
        w = np.asarray(inputs[f"w{li}"], dtype=np.float32)  # [Cout,Cin,k,k]
        CB = (Cin + 127) // 128
        if k == 1:
            wt = w[:, :, 0, 0].T.reshape(CB, 128, Cout)
        else:
            # kb = cb*9 + dy*3 + dx
            wt = (
                w.transpose(1, 2, 3, 0)  # [Cin, k, k, Cout]
                .reshape(CB, 128, k * k, Cout)
                .transpose(0, 2, 1, 3)  # [CB, k*k, 128, Cout]
                .reshape(CB * k * k, 128, Cout)
            )
        # negated: device computes |x + w'| with w' = -w
        maps_common[f"w{li}t"] = np.ascontiguousarray(-wt, dtype=np.float32)
        OB = (Cout + 127) // 128
        g = np.asarray(inputs[f"g{li}"], dtype=np.float32).reshape(OB, 128).T
        b = np.asarray(inputs[f"b{li}"], dtype=np.float32).reshape(OB, 128).T
        maps_common[f"g{li}"] = np.ascontiguousarray(g)
        maps_common[f"b{li}"] = np.ascontiguousarray(b)

    Cin0, _, _, _, _, Hi, Wi, _, _ = shapes[0]
    CB0 = (Cin0 + 127) // 128
    in_maps = []
    n_cores = n_total // N_LOC
    for c in range(n_cores):
        xs = x[c * N_LOC : (c + 1) * N_LOC]  # [2, Cin, H, W]
        xt = (
            xs.transpose(1, 0, 2, 3)
            .reshape(CB0, 128, N_LOC * Hi * Wi)
            .astype(bf16)
        )
        m = dict(maps_common)
        m["xt"] = np.ascontiguousarray(xt)
        in_maps.append(m)
    return in_maps


def _build_nc(shapes=LAYER_SHAPES, n_total=16, n_cores=N_CORES, mock_cc=False):
    import concourse.bacc as bacc
    import concourse.mybir as mybir
    from concourse import tile

    OP = mybir.AluOpType
    AF = mybir.ActivationFunctionType
    BF16 = mybir.dt.bfloat16
    F32 = mybir.dt.float32
    U16 = mybir.dt.uint16

    nc = bacc.Bacc(
        "TRN2",
        target_bir_lowering=False,
        debug=False,
        enable_asserts=False,
        num_devices=n_cores,
    )

    # ---------------- derived per-layer geometry ----------------
    # Activation storage layouts (per layer output), bf16:
    #  L1 -> parity-split padded 40x40 grid: [128, OB, n, ip, jp, 20, 20]
    #  L2 -> plain [128, OB, n*19*19]
    #  L3 -> parity padded 21x21 (ragged) as [128, n, ip, jp, 11, 12]
    #  L4 -> plain [128, OB, n*10*10]
    #  L5 -> parity 10x10 (no pad) as [128, n, ip, jp, 5, 6]
    # final -> fp32 out
    cfg = []
    for li, (Cin, Cout, k, stride, pad, Hi, Wi, Ho, Wo) in enumerate(shapes):
        CB = (Cin + 127) // 128
        OB = (Cout + 127) // 128
        KB = CB * k * k
        nl = N_LOC
        M = nl * Ho * Wo
        cfg.append(
            dict(
                li=li, Cin=Cin, Cout=Cout, k=k, stride=stride, pad=pad,
                Hi=Hi, Wi=Wi, Ho=Ho, Wo=Wo, CB=CB, OB=OB, KB=KB, M=M,
                nl=nl, NM=n_total * Ho * Wo,
            )
        )

    # ---------------- DRAM I/O ----------------
    L1 = cfg[0]
    xt_d = nc.dram_tensor("xt", [L1["CB"], 128, N_LOC * L1["Hi"] * L1["Wi"]], BF16,
                          kind="ExternalInput")
    w_d, g_d, b_d = [], [], []
    for L in cfg:
        li = L["li"] + 1
        w_d.append(nc.dram_tensor(f"w{li}t", [L["KB"], 128, L["Cout"]], F32,
                                  kind="ExternalInput"))
        g_d.append(nc.dram_tensor(f"g{li}", [128, L["OB"]], F32, kind="ExternalInput"))
        b_d.append(nc.dram_tensor(f"b{li}", [128, L["OB"]], F32, kind="ExternalInput"))
    LL = cfg[-1]
    out_d = nc.dram_tensor("out", [N_LOC, LL["Cout"], LL["Ho"], LL["Wo"]], F32,
                           kind="ExternalOutput")

    # ---------------- persistent SBUF ----------------
    def sb(name, cols, dt):
        return nc.alloc_sbuf_tensor(name, [128, cols], dt)

    xsb = sb("xsb", L1["CB"] * N_LOC * L1["Hi"] * L1["Wi"], BF16)

    # activation tensors between layers
    # parity-split geometry helpers: (rows_even_arr, cols) per parity array
    act = []
    act.append(sb("act1p", cfg[0]["OB"] * N_LOC * 2 * 2 * 20 * 20, BF16))
    act.append(sb("act2", cfg[1]["OB"] * N_LOC * 19 * 19, BF16))
    act.append(sb("act3p", N_LOC * 2 * 2 * 11 * 12, BF16))
    act.append(sb("act4", cfg[3]["OB"] * N_LOC * 10 * 10, BF16))
    act.append(sb("act5p", N_LOC * 2 * 2 * 5 * 6, BF16))
    act.append(sb("fin6", cfg[5]["OB"] * N_LOC * 4 * 4, F32))

    S_max = max(L["OB"] * L["M"] for L in cfg)
    S = sb("S", S_max, F32)

    w_sb = [sb(f"wsb{L['li']}", L["KB"] * L["Cout"], F32) for L in cfg]
    g_sb = [sb(f"gsb{L['li']}", L["OB"], F32) for L in cfg]
    b_sb = [sb(f"bsb{L['li']}", L["OB"], F32) for L in cfg]

    ones = sb("ones", 1, BF16)
    # sliding one-hot window: OH[:, 31:32] = 1, rest 0.
    # lhsT for channel row q (0..31) = OH[:, 31-q : 63-q]  -> one-hot col q
    oh = sb("oh", 63, BF16)

    # per-layer stats and BN coefficient tensors (tiny)
    stats_sb, stats_g, abwork = {}, {}, {}
    sqfull = sb("sqfull", max(L["M"] for L in cfg), F32)

    st_in_d, st_out_d = {}, {}

    with tile.TileContext(nc) as tc:
        # ---------------- input loads ----------------
        for cb in range(L1["CB"]):
            W0 = N_LOC * L1["Hi"] * L1["Wi"]
            nc.sync.dma_start(xsb.ap()[:, cb * W0:(cb + 1) * W0], xt_d.ap()[cb])
        for L in cfg:
            li = L["li"]
            nc.sync.dma_start(
                w_sb[li].ap().rearrange("p (k c) -> p k c", k=L["KB"]),
                w_d[li].ap().rearrange("k p c -> p k c"),
            )
            nc.sync.dma_start(g_sb[li].ap(), g_d[li].ap())
            nc.sync.dma_start(b_sb[li].ap(), b_d[li].ap())
        nc.vector.memset(ones.ap(), 1.0)
        nc.vector.memset(oh.ap(), 0.0)
        nc.vector.memset(oh.ap()[:, 31:32], 1.0)
        # zero parity-padded act tensors (borders / pad columns stay zero)
        nc.gpsimd.memset(act[0].ap(), 0.0)
        nc.gpsimd.memset(act[2].ap(), 0.0)
        nc.gpsimd.memset(act[4].ap(), 0.0)

        with (
            tc.tile_pool(name="scr", bufs=8) as scrp,
            tc.tile_pool(name="ps", bufs=8, space="PSUM") as psp,
            tc.tile_pool(name="dram", bufs=2, space="DRAM") as dramp,
        ):
            # ============ patch AP generators ============
            # view helpers over activation tensors
            def patch_and_rhs(L):
                """returns (patch_fn(kb, ph) -> in0 AP matching scratch layout,
                            scr_len, phases, chunk list [(rhs_fn(scr), N, scol)])"""
                li = L["li"]
                k = L["k"]
                if li == 0:
                    W0 = N_LOC * L["Hi"] * L["Wi"]
                    def patch(kb, ph):
                        return xsb.ap()[:, kb * W0:(kb + 1) * W0]
                    # 6 chunks: 5x512 + 328 (PSUM bank limit 512 fp32 cols)
                    bounds = [0, 512, 1024, 1536, 2048, 2560, W0]
                    chunks = [
                        (lambda scr, c=c: scr[:, bounds[c]:bounds[c + 1]],
                         bounds[c + 1] - bounds[c])
                        for c in range(6)
                    ]
                    return patch, W0, 1, chunks
                if k == 1:
                    src = act[li - 1]
                    Mfull = L["M"]  # n*Ho*Wo  (contiguous per cb)
                    def patch(kb, ph):
                        return src.ap()[:, kb * Mfull:(kb + 1) * Mfull]
                    nchunk = max(1, -(-Mfull // 512))
                    while Mfull % nchunk:
                        nchunk += 1
                    cN = Mfull // nchunk
                    chunks = [
                        (lambda scr, c=c: scr[:, c * cN:(c + 1) * cN], cN)
                        for c in range(nchunk)
                    ]
                    return patch, Mfull, 1, chunks
                # 3x3 layers: read parity-split source arrays
                if li == 1:
                    src, CBv, R, C, rows = act[0], L["CB"], 20, 20, 19
                elif li == 3:
                    src, CBv, R, C, rows = act[2], 1, 11, 12, 10
                else:
                    src, CBv, R, C, rows = act[4], 1, 5, 6, 4
                Ho, Wo = L["Ho"], L["Wo"]
                pad = L["pad"]
                nl = L["nl"]

                if CBv > 1:
                    v = src.ap().rearrange(
                        "p (cb n ip jp r c) -> p cb n ip jp r c",
                        cb=CBv, n=nl, ip=2, jp=2, r=R, c=C)
                else:
                    v = src.ap().rearrange(
                        "p (n ip jp r c) -> p n ip jp r c",
                        n=nl, ip=2, jp=2, r=R, c=C)

                def sel(d):
                    # returns (parity_array, start_offset, rhs_col_offset)
                    if pad == 1:
                        return (1, 0, 0) if d == 0 else ((0, 0, 0) if d == 1 else (1, 1, 1))
                    return (0, 0, 0) if d == 0 else ((1, 0, 0) if d == 1 else (0, 1, 1))

                def patch(kb, ph):
                    cb, r9 = divmod(kb, 9)
                    dy, dx = divmod(r9, 3)
                    ipv, rs, _ = sel(dy)
                    jpv, _cs0, _ = sel(dx)
                    # full padded-width rows for alignment; rhs skips columns
                    if CBv > 1:
                        return v[:, cb, :, ipv, jpv, rs:rs + Ho, 0:C]
                    return v[:, :, ipv, jpv, rs:rs + Ho, 0:C]

                def rhs_cs(kb):
                    _, r9 = divmod(kb, 9)
                    _dy, dx = divmod(r9, 3)
                    return sel(dx)[2]

                scr_len = nl * Ho * C
                if li == 1:
                    # one chunk per image: [19,19] of the [19,20] rows
                    def mk(nn):
                        def f(scr, kb):
                            cs = rhs_cs(kb)
                            sv = scr.rearrange("p (n r c) -> p n r c",
                                               n=nl, r=Ho, c=C)
                            return sv[:, nn, :, cs:cs + Wo]
                        return f
                    chunks = [(mk(nn), Ho * Wo) for nn in range(nl)]
                else:
                    def f(scr, kb):
                        cs = rhs_cs(kb)
                        sv = scr.rearrange("p (n r c) -> p n r c",
                                           n=nl, r=Ho, c=C)
                        return sv[:, :, :, cs:cs + Wo]
                    chunks = [(f, nl * Ho * Wo)]
                return patch, scr_len, 1, chunks

            # ============ conv layers ============
            ACT_FRAC = 0.45  # fraction of abs-diff tiles on ScalarE
            eng_acc = [0.0]
            for L in cfg:
                li, Cout, KB, M, OB = L["li"], L["Cout"], L["KB"], L["M"], L["OB"]
                patch, scr_len, phases, chunks = patch_and_rhs(L)
                nchunk = len(chunks)
                stats_sb[li] = sb(f"stats{li}", 2 * OB, F32)
                stats_g[li] = sb(f"statsg{li}", 2 * OB, F32)
                abwork[li] = sb(f"abw{li}", 12 * OB, F32)

                Ho, Wo = L["Ho"], L["Wo"]

                def stats_ob(ob, li=li, M=M):
                    # per-block stats accumulation; overlaps next block's conv
                    nc.vector.tensor_scalar(
                        sqfull.ap()[:, :M], S.ap()[:, ob * M:(ob + 1) * M],
                        0.0, 0.0, op0=OP.add, op1=OP.add,
                        accum_out=stats_sb[li].ap()[:, 2 * ob:2 * ob + 1],
                    )
                    nc.scalar.activation(
                        sqfull.ap()[:, :M], S.ap()[:, ob * M:(ob + 1) * M],
                        AF.Square,
                        accum_out=stats_sb[li].ap()[:, 2 * ob + 1:2 * ob + 2],
                    )

                def layer_allreduce(li=li, OB=OB):
                    # one AllReduce of all blocks' (sum, sumsq) for this layer
                    sti = dramp.tile([128, 2 * OB], F32, tag=f"sti{li}",
                                     name=f"sti{li}")
                    sto = dramp.tile([128, 2 * OB], F32, tag=f"sto{li}",
                                     name=f"sto{li}")
                    nc.sync.dma_start(sti[:, :], stats_sb[li].ap())
                    if mock_cc:
                        nc.sync.dma_start(sto[:, :], sti[:, :])
                    else:
                        nc.gpsimd.collective_compute(
                            "AllReduce", OP.add,
                            replica_groups=[list(range(n_cores))],
                            ins=[sti.opt()], outs=[sto.opt()],
                        )
                    nc.sync.dma_start(stats_g[li].ap(), sto[:, :])

                def finish_ob(ob, li=li, L=L, M=M, OB=OB, Ho=Ho, Wo=Wo):
                    # BN coeffs from global stats -> epilogue
                    aw = abwork[li].ap()

                    def col(i):
                        return aw[:, i * OB + ob:i * OB + ob + 1]

                    (mean, ex2, m2, vpe, sq_, rc, u, s2, rinv, ga, negA,
                     Bv) = [col(i) for i in range(12)]
                    inm = 1.0 / L["NM"]
                    nc.vector.tensor_scalar_mul(
                        mean, stats_g[li].ap()[:, 2 * ob:2 * ob + 1], inm)
                    nc.vector.tensor_scalar_mul(
                        ex2, stats_g[li].ap()[:, 2 * ob + 1:2 * ob + 2], inm)
                    nc.scalar.activation(m2, mean, AF.Square)
                    nc.vector.tensor_tensor(vpe, ex2, m2, op=OP.subtract)
                    nc.vector.tensor_scalar_add(vpe, vpe, EPS)
                    nc.scalar.activation(sq_, vpe, AF.Sqrt)
                    nc.vector.reciprocal(rc, sq_)
                    nc.vector.tensor_tensor(u, vpe, rc, op=OP.mult)
                    nc.vector.tensor_tensor(s2, sq_, u, op=OP.add)
                    nc.vector.tensor_scalar_mul(s2, s2, 0.5)
                    nc.vector.reciprocal(rinv, s2)
                    gcol = g_sb[li].ap()[:, ob:ob + 1]
                    bcol = b_sb[li].ap()[:, ob:ob + 1]
                    nc.vector.tensor_tensor(ga, gcol, rinv, op=OP.mult)
                    nc.vector.tensor_scalar_mul(negA, ga, -1.0)
                    nc.vector.tensor_tensor(Bv, mean, ga, op=OP.mult)
                    nc.vector.tensor_tensor(Bv, bcol, Bv, op=OP.add)

                    def apply_tsp(dst_ap, src_ap):
                        nc.vector.tensor_scalar(
                            dst_ap, src_ap, negA, Bv, OP.mult, OP.add)

                    if li in (0, 2, 4):
                        if li == 0:
                            R, C = 20, 20
                            dstv = act[0].ap().rearrange(
                                "p (obb n ip jp r c) -> p obb n ip jp r c",
                                obb=OB, n=N_LOC, ip=2, jp=2, r=R, c=C)
                        elif li == 2:
                            R, C = 11, 12
                            dstv = act[2].ap().rearrange(
                                "p (n ip jp r c) -> p n ip jp r c",
                                n=N_LOC, ip=2, jp=2, r=R, c=C)
                        else:
                            R, C = 5, 6
                            dstv = act[4].ap().rearrange(
                                "p (n ip jp r c) -> p n ip jp r c",
                                n=N_LOC, ip=2, jp=2, r=R, c=C)
                        Sv = S.ap()[:, :OB * M].rearrange(
                            "p (obb n i j) -> p obb n i j",
                            obb=OB, n=N_LOC, i=Ho, j=Wo)
                        pad = 1 if li in (0, 2) else 0
                        for bpar in (0, 1):
                            for dpar in (0, 1):
                                srcv = Sv[:, ob, :, bpar::2, dpar::2]
                                na, ncc = srcv.shape[2], srcv.shape[3]
                                if pad == 1:
                                    ipv, rs = (0, 0) if bpar == 0 else (1, 1)
                                    jpv, cs = (0, 0) if dpar == 0 else (1, 1)
                                else:
                                    ipv, rs = (0, 0) if bpar == 0 else (1, 0)
                                    jpv, cs = (0, 0) if dpar == 0 else (1, 0)
                                if li == 0:
                                    dst = dstv[:, ob, :, ipv, jpv,
                                               rs:rs + na, cs:cs + ncc]
                                else:
                                    dst = dstv[:, :, ipv, jpv,
                                               rs:rs + na, cs:cs + ncc]
                                apply_tsp(dst, srcv)
                        blk = act[li].ap().shape[1] // OB
                        tgt = act[li].ap()[:, ob * blk:(ob + 1) * blk]
                        nc.vector.tensor_scalar(tgt, tgt, 0.0, 6.0,
                                                OP.max, OP.min)
                    else:
                        dst_t = act[li] if li != 5 else act[5]
                        apply_tsp(dst_t.ap()[:, ob * M:(ob + 1) * M],
                                  S.ap()[:, ob * M:(ob + 1) * M])
                        tgt = dst_t.ap()[:, ob * M:(ob + 1) * M]
                        nc.vector.tensor_scalar(tgt, tgt, 0.0, 6.0,
                                                OP.max, OP.min)

                for ob in range(OB):
                    pss = [
                        psp.tile([128, chunks[c][1]], F32, tag="ps",
                                 name=f"ps{li}_{c}",
                                 padded_shape=[128, 512])
                        for c in range(nchunk)
                    ]
                    for kb in range(KB):
                        for o_r in range(128):
                            o = ob * 128 + o_r
                            quad, q = divmod(o_r, 32)
                            scr = scrp.tile([128, scr_len], BF16, tag="scr")
                            wap = w_sb[li].ap()[:, kb * Cout + o:kb * Cout + o + 1]
                            eng_acc[0] += ACT_FRAC
                            if eng_acc[0] >= 1.0:
                                eng_acc[0] -= 1.0
                                nc.scalar.activation(
                                    scr[:, :scr_len], patch(kb, 0), AF.Abs,
                                    bias=wap, scale=1.0)
                            else:
                                nc.vector.tensor_scalar_add(
                                    scr[:, :scr_len], patch(kb, 0), wap)
                                u = scr[:, :scr_len].bitcast(U16)
                                nc.vector.tensor_scalar(
                                    u, u, 0x7FFF, None,
                                    OP.bitwise_and, OP.bypass)
                            for c in range(nchunk):
                                rhs_fn, cN = chunks[c]
                                rhs = (rhs_fn(scr, kb) if L["k"] == 3
                                       else rhs_fn(scr))
                                nc.tensor.matmul(
                                    pss[c][32 * quad:32 * quad + 32, :],
                                    oh.ap()[:, 31 - q:63 - q], rhs,
                                    start=(q == 0 and kb == 0),
                                    stop=(q == 31 and kb == KB - 1),
                                    tile_position=(0, 32 * quad),
                                    skip_group_check=True,
                                )
                    # evacuate whole 128-channel block -> S (aligned)
                    coff = 0
                    for c in range(nchunk):
                        cN = chunks[c][1]
                        scol = ob * M + coff
                        nc.vector.tensor_scalar_add(
                            S.ap()[:, scol:scol + cN], pss[c][:, :cN], 0.0,
                        )
                        coff += cN
                    stats_ob(ob)
                layer_allreduce()
                for ob in range(OB):
                    finish_ob(ob)

            # ---------------- final output DMA ----------------
            hw = LL["Ho"] * LL["Wo"]
            finv = act[5].ap().rearrange("p (ob n hw) -> p ob n hw",
                                         ob=LL["OB"], n=N_LOC, hw=hw)
            dst = out_d.ap().rearrange("n (ob p) h w -> p ob n (h w)",
                                       ob=LL["OB"], p=128)
            for ob in range(LL["OB"]):
                nc.sync.dma_start(dst[:, ob], finv[:, ob])

    nc.compile()
    return nc


def _get_nc():
    if "nc" not in _NC_CACHE:
        _NC_CACHE["nc"] = _build_nc()
    return _NC_CACHE["nc"]


def kernel(**inputs) -> np.ndarray:
    import time as _time
    from concourse.bass_utils import run_bass_kernel_spmd

    nc = _get_nc()
    in_maps = _host_prep(inputs)
    last = None
    for attempt in range(3):
        try:
            res = run_bass_kernel_spmd(nc, in_maps, core_ids=list(range(N_CORES)))
            outs = [np.asarray(r["out"]).reshape(N_LOC, 256, 4, 4)
                    for r in res.results]
            return np.concatenate(outs, axis=0).astype(np.float32)
        except Exception as e:  # transient axon mesh desync: wait + retry
            last = e
            _time.sleep(20 * (attempt + 1))
    raise last


if __name__ == "__main__":
    nc = _build_nc()
    print("build + compile OK")



# revision 30
# speedup vs baseline: 1.4106x; 1.0052x over previous
"""AdderNet CNN (6x adder_conv + sync-BN + ReLU6) on 8 Trainium2 NeuronCores.

Sharding: data-parallel over batch (2 images/core), sync-BN via ONE
AllReduce of per-channel (sum, sumsq) per layer (6 total; per-block
collectives serialize the pipeline and cost ~20ms extra wall).

Per-layer compute (per core):
  - contraction dim k lives on SBUF partitions (128-sized k-blocks)
  - abs-diff feed split ~45/55 between ScalarE (activation Abs with
    per-partition bias = -w, ~151 G elem/s) and VectorE (tensor_scalar
    add + uint16 bitwise_and 0x7FFF sign-clear, 2 passes at ~371 G
    elem-visits/s), both writing bf16 scratch tiles
  - k-partition-sum via TensorE matmul with a sliding one-hot [128,32]
    stationary matrix: channel o accumulates into PSUM row o%128 of a
    shared per-chunk tile (PE array tiling tile_position=(0,32*quad);
    first matmul of each 32-row quadrant resets it via start=True), so a
    full 128-channel block evacuates with one aligned [128,N] DVE op
  - per-channel sum/sumsq via tensor_scalar/activation(Square) accum_out,
    AllReduce of stats (sync-BN), then BN+ReLU6 via tensor_scalar
    (mult,add then max,min), writing bf16 activations in parity-split
    padded layouts so stride-2 consumers read unit-stride, even-length,
    4B-aligned rows (keeps DVE 4x / clean PE rhs APs).
"""

import sys
import numpy as np

if "/opt/trn_rl_repo" not in sys.path:
    sys.path.insert(0, "/opt/trn_rl_repo")

import ml_dtypes

N_CORES = 8
N_LOC = 2  # images per core
EPS = 1e-5

# (Cin, Cout, k, stride, pad, Hi, Wi, Ho, Wo)
LAYER_SHAPES = [
    (512, 256, 1, 1, 0, 38, 38, 38, 38),
    (256, 512, 3, 2, 1, 38, 38, 19, 19),
    (512, 128, 1, 1, 0, 19, 19, 19, 19),
    (128, 256, 3, 2, 1, 19, 19, 10, 10),
    (256, 128, 1, 1, 0, 10, 10, 10, 10),
    (128, 256, 3, 2, 0, 10, 10, 4, 4),
]

_NC_CACHE = {}


def _host_prep(inputs, shapes=LAYER_SHAPES, n_total=16):
    """Build per-core in_maps from the raw reference inputs."""
    bf16 = ml_dtypes.bfloat16
    x = np.asarray(inputs["x"], dtype=np.float32)
    maps_common = {}
    for li, (Cin, Cout, k, *_rest) in enumerate(shapes, start=1):
        w = np.asarray(inputs[f"w{li}"], dtype=np.float32)  # [Cout,Cin,k,k]
        CB = (Cin + 127) // 128
        if k == 1:
            wt = w[:, :, 0, 0].T.reshape(CB, 128, Cout)
        else:
            # kb = cb*9 + dy*3 + dx
            wt = (
                w.transpose(1, 2, 3, 0)  # [Cin, k, k, Cout]
                .reshape(CB, 128, k * k, Cout)
                .transpose(0, 2, 1, 3)  # [CB, k*k, 128, Cout]
                .reshape(CB * k * k, 128, Cout)
            )
        # negated: device computes |x + w'| with w' = -w
        maps_common[f"w{li}t"] = np.ascontiguousarray(-wt, dtype=np.float32)
        OB = (Cout + 127) // 128
        g = np.asarray(inputs[f"g{li}"], dtype=np.float32).reshape(OB, 128).T
        b = np.asarray(inputs[f"b{li}"], dtype=np.float32).reshape(OB, 128).T
        maps_common[f"g{li}"] = np.ascontiguousarray(g)
        maps_common[f"b{li}"] = np.ascontiguousarray(b)

    Cin0, _, _, _, _, Hi, Wi, _, _ = shapes[0]
    CB0 = (Cin0 + 127) // 128
    in_maps = []
    n_cores = n_total // N_LOC
    for c in range(n_cores):
        xs = x[c * N_LOC : (c + 1) * N_LOC]  # [2, Cin, H, W]
        xt = (
            xs.transpose(1, 0, 2, 3)
            .reshape(CB0, 128, N_LOC * Hi * Wi)
            .astype(bf16)
        )
        m = dict(maps_common)
        m["xt"] = np.ascontiguousarray(xt)
        in_maps.append(m)
    return in_maps


def _build_nc(shapes=LAYER_SHAPES, n_total=16, n_cores=N_CORES, mock_cc=False,
              split_cc=False, scr_bufs=10):
    import concourse.bacc as bacc
    import concourse.mybir as mybir
    from concourse import tile

    OP = mybir.AluOpType
    AF = mybir.ActivationFunctionType
    BF16 = mybir.dt.bfloat16
    F32 = mybir.dt.float32
    U16 = mybir.dt.uint16

    nc = bacc.Bacc(
        "TRN2",
        target_bir_lowering=False,
        debug=False,
        enable_asserts=False,
        num_devices=n_cores,
    )

    # ---------------- derived per-layer geometry ----------------
    # Activation storage layouts (per layer output), bf16:
    #  L1 -> parity-split padded 40x40 grid: [128, OB, n, ip, jp, 20, 20]
    #  L2 -> plain [128, OB, n*19*19]
    #  L3 -> parity padded 21x21 (ragged) as [128, n, ip, jp, 11, 12]
    #  L4 -> plain [128, OB, n*10*10]
    #  L5 -> parity 10x10 (no pad) as [128, n, ip, jp, 5, 6]
    # final -> fp32 out
    cfg = []
    for li, (Cin, Cout, k, stride, pad, Hi, Wi, Ho, Wo) in enumerate(shapes):
        CB = (Cin + 127) // 128
        OB = (Cout + 127) // 128
        KB = CB * k * k
        nl = N_LOC
        M = nl * Ho * Wo
        cfg.append(
            dict(
                li=li, Cin=Cin, Cout=Cout, k=k, stride=stride, pad=pad,
                Hi=Hi, Wi=Wi, Ho=Ho, Wo=Wo, CB=CB, OB=OB, KB=KB, M=M,
                nl=nl, NM=n_total * Ho * Wo,
            )
        )

    # ---------------- DRAM I/O ----------------
    L1 = cfg[0]
    xt_d = nc.dram_tensor("xt", [L1["CB"], 128, N_LOC * L1["Hi"] * L1["Wi"]], BF16,
                          kind="ExternalInput")
    w_d, g_d, b_d = [], [], []
    for L in cfg:
        li = L["li"] + 1
        w_d.append(nc.dram_tensor(f"w{li}t", [L["KB"], 128, L["Cout"]], F32,
                                  kind="ExternalInput"))
        g_d.append(nc.dram_tensor(f"g{li}", [128, L["OB"]], F32, kind="ExternalInput"))
        b_d.append(nc.dram_tensor(f"b{li}", [128, L["OB"]], F32, kind="ExternalInput"))
    LL = cfg[-1]
    out_d = nc.dram_tensor("out", [N_LOC, LL["Cout"], LL["Ho"], LL["Wo"]], F32,
                           kind="ExternalOutput")

    # ---------------- persistent SBUF ----------------
    def sb(name, cols, dt):
        return nc.alloc_sbuf_tensor(name, [128, cols], dt)

    xsb = sb("xsb", L1["CB"] * N_LOC * L1["Hi"] * L1["Wi"], BF16)

    # activation tensors between layers
    # parity-split geometry helpers: (rows_even_arr, cols) per parity array
    act = []
    act.append(sb("act1p", cfg[0]["OB"] * N_LOC * 2 * 2 * 20 * 20, BF16))
    act.append(sb("act2", cfg[1]["OB"] * N_LOC * 19 * 19, BF16))
    act.append(sb("act3p", N_LOC * 2 * 2 * 11 * 12, BF16))
    act.append(sb("act4", cfg[3]["OB"] * N_LOC * 10 * 10, BF16))
    act.append(sb("act5p", N_LOC * 2 * 2 * 5 * 6, BF16))
    act.append(sb("fin6", cfg[5]["OB"] * N_LOC * 4 * 4, F32))

    S_max = max(L["OB"] * L["M"] for L in cfg)
    S = sb("S", S_max, F32)

    w_sb = [sb(f"wsb{L['li']}", L["KB"] * L["Cout"], F32) for L in cfg]
    g_sb = [sb(f"gsb{L['li']}", L["OB"], F32) for L in cfg]
    b_sb = [sb(f"bsb{L['li']}", L["OB"], F32) for L in cfg]

    ones = sb("ones", 1, BF16)
    # sliding one-hot window: OH[:, 31:32] = 1, rest 0.
    # lhsT for channel row q (0..31) = OH[:, 31-q : 63-q]  -> one-hot col q
    oh = sb("oh", 63, BF16)

    # per-layer stats and BN coefficient tensors (tiny)
    stats_sb, stats_g, abwork = {}, {}, {}
    sqfull = sb("sqfull", max(L["M"] for L in cfg), F32)

    st_in_d, st_out_d = {}, {}

    with tile.TileContext(nc) as tc:
        # ---------------- input loads ----------------
        for cb in range(L1["CB"]):
            W0 = N_LOC * L1["Hi"] * L1["Wi"]
            nc.sync.dma_start(xsb.ap()[:, cb * W0:(cb + 1) * W0], xt_d.ap()[cb])
        for L in cfg:
            li = L["li"]
            nc.sync.dma_start(
                w_sb[li].ap().rearrange("p (k c) -> p k c", k=L["KB"]),
                w_d[li].ap().rearrange("k p c -> p k c"),
            )
            nc.sync.dma_start(g_sb[li].ap(), g_d[li].ap())
            nc.sync.dma_start(b_sb[li].ap(), b_d[li].ap())
        nc.vector.memset(ones.ap(), 1.0)
        nc.vector.memset(oh.ap(), 0.0)
        nc.vector.memset(oh.ap()[:, 31:32], 1.0)
        # zero parity-padded act tensors (borders / pad columns stay zero)
        nc.gpsimd.memset(act[0].ap(), 0.0)
        nc.gpsimd.memset(act[2].ap(), 0.0)
        nc.gpsimd.memset(act[4].ap(), 0.0)

        with (
            tc.tile_pool(name="scr", bufs=scr_bufs) as scrp,
            tc.tile_pool(name="ps", bufs=8, space="PSUM") as psp,
            tc.tile_pool(name="dram", bufs=2, space="DRAM") as dramp,
        ):
            # ============ patch AP generators ============
            # view helpers over activation tensors
            def patch_and_rhs(L):
                """returns (patch_fn(kb, ph) -> in0 AP matching scratch layout,
                            scr_len, phases, chunk list [(rhs_fn(scr), N, scol)])"""
                li = L["li"]
                k = L["k"]
                if li == 0:
                    W0 = N_LOC * L["Hi"] * L["Wi"]
                    def patch(kb, ph):
                        return xsb.ap()[:, kb * W0:(kb + 1) * W0]
                    # 6 chunks: 5x512 + 328 (PSUM bank limit 512 fp32 cols)
                    bounds = [0, 512, 1024, 1536, 2048, 2560, W0]
                    chunks = [
                        (lambda scr, c=c: scr[:, bounds[c]:bounds[c + 1]],
                         bounds[c + 1] - bounds[c])
                        for c in range(6)
                    ]
                    return patch, W0, 1, chunks
                if k == 1:
                    src = act[li - 1]
                    Mfull = L["M"]  # n*Ho*Wo  (contiguous per cb)
                    def patch(kb, ph):
                        return src.ap()[:, kb * Mfull:(kb + 1) * Mfull]
                    nchunk = max(1, -(-Mfull // 512))
                    while Mfull % nchunk:
                        nchunk += 1
                    cN = Mfull // nchunk
                    chunks = [
                        (lambda scr, c=c: scr[:, c * cN:(c + 1) * cN], cN)
                        for c in range(nchunk)
                    ]
                    return patch, Mfull, 1, chunks
                # 3x3 layers: read parity-split source arrays
                if li == 1:
                    src, CBv, R, C, rows = act[0], L["CB"], 20, 20, 19
                elif li == 3:
                    src, CBv, R, C, rows = act[2], 1, 11, 12, 10
                else:
                    src, CBv, R, C, rows = act[4], 1, 5, 6, 4
                Ho, Wo = L["Ho"], L["Wo"]
                pad = L["pad"]
                nl = L["nl"]

                if CBv > 1:
                    v = src.ap().rearrange(
                        "p (cb n ip jp r c) -> p cb n ip jp r c",
                        cb=CBv, n=nl, ip=2, jp=2, r=R, c=C)
                else:
                    v = src.ap().rearrange(
                        "p (n ip jp r c) -> p n ip jp r c",
                        n=nl, ip=2, jp=2, r=R, c=C)

                def sel(d):
                    # returns (parity_array, start_offset, rhs_col_offset)
                    if pad == 1:
                        return (1, 0, 0) if d == 0 else ((0, 0, 0) if d == 1 else (1, 1, 1))
                    return (0, 0, 0) if d == 0 else ((1, 0, 0) if d == 1 else (0, 1, 1))

                def patch(kb, ph):
                    cb, r9 = divmod(kb, 9)
                    dy, dx = divmod(r9, 3)
                    ipv, rs, _ = sel(dy)
                    jpv, _cs0, _ = sel(dx)
                    # full padded-width rows for alignment; rhs skips columns
                    if CBv > 1:
                        return v[:, cb, :, ipv, jpv, rs:rs + Ho, 0:C]
                    return v[:, :, ipv, jpv, rs:rs + Ho, 0:C]

                def rhs_cs(kb):
                    _, r9 = divmod(kb, 9)
                    _dy, dx = divmod(r9, 3)
                    return sel(dx)[2]

                scr_len = nl * Ho * C
                if li == 1:
                    # one chunk per image: [19,19] of the [19,20] rows
                    def mk(nn):
                        def f(scr, kb):
                            cs = rhs_cs(kb)
                            sv = scr.rearrange("p (n r c) -> p n r c",
                                               n=nl, r=Ho, c=C)
                            return sv[:, nn, :, cs:cs + Wo]
                        return f
                    chunks = [(mk(nn), Ho * Wo) for nn in range(nl)]
                else:
                    def f(scr, kb):
                        cs = rhs_cs(kb)
                        sv = scr.rearrange("p (n r c) -> p n r c",
                                           n=nl, r=Ho, c=C)
                        return sv[:, :, :, cs:cs + Wo]
                    chunks = [(f, nl * Ho * Wo)]
                return patch, scr_len, 1, chunks

            # ============ conv layers ============
            ACT_FRAC = 0.45  # fraction of abs-diff tiles on ScalarE
            eng_acc = [0.0]
            for L in cfg:
                li, Cout, KB, M, OB = L["li"], L["Cout"], L["KB"], L["M"], L["OB"]
                patch, scr_len, phases, chunks = patch_and_rhs(L)
                nchunk = len(chunks)
                stats_sb[li] = sb(f"stats{li}", 2 * OB, F32)
                stats_g[li] = sb(f"statsg{li}", 2 * OB, F32)
                abwork[li] = sb(f"abw{li}", 12 * OB, F32)

                Ho, Wo = L["Ho"], L["Wo"]

                def stats_ob(ob, li=li, M=M):
                    # per-block stats accumulation; overlaps next block's conv
                    nc.vector.tensor_scalar(
                        sqfull.ap()[:, :M], S.ap()[:, ob * M:(ob + 1) * M],
                        0.0, 0.0, op0=OP.add, op1=OP.add,
                        accum_out=stats_sb[li].ap()[:, 2 * ob:2 * ob + 1],
                    )
                    nc.scalar.activation(
                        sqfull.ap()[:, :M], S.ap()[:, ob * M:(ob + 1) * M],
                        AF.Square,
                        accum_out=stats_sb[li].ap()[:, 2 * ob + 1:2 * ob + 2],
                    )

                def layer_allreduce(ob0, ob1, li=li):
                    # AllReduce of blocks [ob0, ob1)'s (sum, sumsq)
                    sti = dramp.tile([128, 2 * (ob1 - ob0)], F32,
                                     tag=f"sti{li}_{ob0}", name=f"sti{li}_{ob0}")
                    sto = dramp.tile([128, 2 * (ob1 - ob0)], F32,
                                     tag=f"sto{li}_{ob0}", name=f"sto{li}_{ob0}")
                    nc.sync.dma_start(sti[:, :],
                                      stats_sb[li].ap()[:, 2 * ob0:2 * ob1])
                    if mock_cc:
                        nc.sync.dma_start(sto[:, :], sti[:, :])
                    else:
                        nc.gpsimd.collective_compute(
                            "AllReduce", OP.add,
                            replica_groups=[list(range(n_cores))],
                            ins=[sti.opt()], outs=[sto.opt()],
                        )
                    nc.sync.dma_start(stats_g[li].ap()[:, 2 * ob0:2 * ob1],
                                      sto[:, :])

                def finish_ob(ob, li=li, L=L, M=M, OB=OB, Ho=Ho, Wo=Wo):
                    # BN coeffs from global stats -> epilogue
                    aw = abwork[li].ap()

                    def col(i):
                        return aw[:, i * OB + ob:i * OB + ob + 1]

                    (mean, ex2, m2, vpe, sq_, rc, u, s2, rinv, ga, negA,
                     Bv) = [col(i) for i in range(12)]
                    inm = 1.0 / L["NM"]
                    nc.vector.tensor_scalar_mul(
                        mean, stats_g[li].ap()[:, 2 * ob:2 * ob + 1], inm)
                    nc.vector.tensor_scalar_mul(
                        ex2, stats_g[li].ap()[:, 2 * ob + 1:2 * ob + 2], inm)
                    nc.scalar.activation(m2, mean, AF.Square)
                    nc.vector.tensor_tensor(vpe, ex2, m2, op=OP.subtract)
                    nc.vector.tensor_scalar_add(vpe, vpe, EPS)
                    nc.scalar.activation(sq_, vpe, AF.Sqrt)
                    nc.vector.reciprocal(rc, sq_)
                    nc.vector.tensor_tensor(u, vpe, rc, op=OP.mult)
                    nc.vector.tensor_tensor(s2, sq_, u, op=OP.add)
                    nc.vector.tensor_scalar_mul(s2, s2, 0.5)
                    nc.vector.reciprocal(rinv, s2)
                    gcol = g_sb[li].ap()[:, ob:ob + 1]
                    bcol = b_sb[li].ap()[:, ob:ob + 1]
                    nc.vector.tensor_tensor(ga, gcol, rinv, op=OP.mult)
                    nc.vector.tensor_scalar_mul(negA, ga, -1.0)
                    nc.vector.tensor_tensor(Bv, mean, ga, op=OP.mult)
                    nc.vector.tensor_tensor(Bv, bcol, Bv, op=OP.add)

                    def apply_tsp(dst_ap, src_ap):
                        nc.vector.tensor_scalar(
                            dst_ap, src_ap, negA, Bv, OP.mult, OP.add)

                    if li in (0, 2, 4):
                        if li == 0:
                            R, C = 20, 20
                            dstv = act[0].ap().rearrange(
                                "p (obb n ip jp r c) -> p obb n ip jp r c",
                                obb=OB, n=N_LOC, ip=2, jp=2, r=R, c=C)
                        elif li == 2:
                            R, C = 11, 12
                            dstv = act[2].ap().rearrange(
                                "p (n ip jp r c) -> p n ip jp r c",
                                n=N_LOC, ip=2, jp=2, r=R, c=C)
                        else:
                            R, C = 5, 6
                            dstv = act[4].ap().rearrange(
                                "p (n ip jp r c) -> p n ip jp r c",
                                n=N_LOC, ip=2, jp=2, r=R, c=C)
                        Sv = S.ap()[:, :OB * M].rearrange(
                            "p (obb n i j) -> p obb n i j",
                            obb=OB, n=N_LOC, i=Ho, j=Wo)
                        pad = 1 if li in (0, 2) else 0
                        for bpar in (0, 1):
                            for dpar in (0, 1):
                                srcv = Sv[:, ob, :, bpar::2, dpar::2]
                                na, ncc = srcv.shape[2], srcv.shape[3]
                                if pad == 1:
                                    ipv, rs = (0, 0) if bpar == 0 else (1, 1)
                                    jpv, cs = (0, 0) if dpar == 0 else (1, 1)
                                else:
                                    ipv, rs = (0, 0) if bpar == 0 else (1, 0)
                                    jpv, cs = (0, 0) if dpar == 0 else (1, 0)
                                if li == 0:
                                    dst = dstv[:, ob, :, ipv, jpv,
                                               rs:rs + na, cs:cs + ncc]
                                else:
                                    dst = dstv[:, :, ipv, jpv,
                                               rs:rs + na, cs:cs + ncc]
                                apply_tsp(dst, srcv)
                        blk = act[li].ap().shape[1] // OB
                        tgt = act[li].ap()[:, ob * blk:(ob + 1) * blk]
                        nc.vector.tensor_scalar(tgt, tgt, 0.0, 6.0,
                                                OP.max, OP.min)
                    else:
                        dst_t = act[li] if li != 5 else act[5]
                        apply_tsp(dst_t.ap()[:, ob * M:(ob + 1) * M],
                                  S.ap()[:, ob * M:(ob + 1) * M])
                        tgt = dst_t.ap()[:, ob * M:(ob + 1) * M]
                        nc.vector.tensor_scalar(tgt, tgt, 0.0, 6.0,
                                                OP.max, OP.min)

                for ob in range(OB):
                    pss = [
                        psp.tile([128, chunks[c][1]], F32, tag="ps",
                                 name=f"ps{li}_{c}",
                                 padded_shape=[128, 512])
                        for c in range(nchunk)
                    ]
                    for kb in range(KB):
                        for o_r in range(128):
                            o = ob * 128 + o_r
                            quad, q = divmod(o_r, 32)
                            scr = scrp.tile([128, scr_len], BF16, tag="scr")
                            wap = w_sb[li].ap()[:, kb * Cout + o:kb * Cout + o + 1]
                            eng_acc[0] += ACT_FRAC
                            if eng_acc[0] >= 1.0:
                                eng_acc[0] -= 1.0
                                nc.scalar.activation(
                                    scr[:, :scr_len], patch(kb, 0), AF.Abs,
                                    bias=wap, scale=1.0)
                            else:
                                nc.vector.tensor_scalar_add(
                                    scr[:, :scr_len], patch(kb, 0), wap)
                                u = scr[:, :scr_len].bitcast(U16)
                                nc.vector.tensor_scalar(
                                    u, u, 0x7FFF, None,
                                    OP.bitwise_and, OP.bypass)
                            for c in range(nchunk):
                                rhs_fn, cN = chunks[c]
                                rhs = (rhs_fn(scr, kb) if L["k"] == 3
                                       else rhs_fn(scr))
                                nc.tensor.matmul(
                                    pss[c][32 * quad:32 * quad + 32, :],
                                    oh.ap()[:, 31 - q:63 - q], rhs,
                                    start=(q == 0 and kb == 0),
                                    stop=(q == 31 and kb == KB - 1),
                                    tile_position=(0, 32 * quad),
                                    skip_group_check=True,
                                )
                    # evacuate whole 128-channel block -> S (aligned)
                    coff = 0
                    for c in range(nchunk):
                        cN = chunks[c][1]
                        scol = ob * M + coff
                        nc.vector.tensor_scalar_add(
                            S.ap()[:, scol:scol + cN], pss[c][:, :cN], 0.0,
                        )
                        coff += cN
                    stats_ob(ob)
                    if split_cc and OB > 1 and ob == OB - 2:
                        # early collective for blocks 0..OB-2: flies while the
                        # last block's conv runs, so next layer's first
                        # k-blocks (which need only these channels) can start
                        # before the final block's stats round-trip lands
                        layer_allreduce(0, OB - 1)
                if split_cc and OB > 1:
                    layer_allreduce(OB - 1, OB)
                else:
                    layer_allreduce(0, OB)
                for ob in range(OB):
                    finish_ob(ob)

            # ---------------- final output DMA ----------------
            hw = LL["Ho"] * LL["Wo"]
            finv = act[5].ap().rearrange("p (ob n hw) -> p ob n hw",
                                         ob=LL["OB"], n=N_LOC, hw=hw)
            dst = out_d.ap().rearrange("n (ob p) h w -> p ob n (h w)",
                                       ob=LL["OB"], p=128)
            for ob in range(LL["OB"]):
                nc.sync.dma_start(dst[:, ob], finv[:, ob])

    nc.compile()
    return nc


def _get_nc():
    if "nc" not in _NC_CACHE:
        _NC_CACHE["nc"] = _build_nc()
    return _NC_CACHE["nc"]


def kernel(**inputs) -> np.ndarray:
    import time as _time
    from concourse.bass_utils import run_bass_kernel_spmd

    nc = _get_nc()
    in_maps = _host_prep(inputs)
    last = None
    for attempt in range(3):
        try:
            res = run_bass_kernel_spmd(nc, in_maps, core_ids=list(range(N_CORES)))
            outs = [np.asarray(r["out"]).reshape(N_LOC, 256, 4, 4)
                    for r in res.results]
            return np.concatenate(outs, axis=0).astype(np.float32)
        except Exception as e:  # transient axon mesh desync: wait + retry
            last = e
            _time.sleep(20 * (attempt + 1))
    raise last


if __name__ == "__main__":
    nc = _build_nc()
    print("build + compile OK")

